# revision 2
# baseline (speedup 1.0000x reference)
"""AxileAttention Trainium2 kernel (self-contained).

Problem: x[8,64,256,256] fp32; per-channel weights *_w[64,256,256], biases *_b[64,256,256]:
    q = einsum("bchw,cwv->bchv", x, query_w) + query_b   (same for k with key_*, v with var_*)
    out = softmax(q*k, axis=-1) * v

Strategy (8 NeuronCores, SPMD via run_bass_kernel_spmd):
  * Shard the channel axis C=64 -> 8 channels/core; weights+biases sharded the same
    way, x sliced per core. Each core computes 64 (batch, channel) pairs.
  * Host pre-transposes x to xT[b,cc,w,h'] (h' in the interleaved order h = 2j+m)
    so the contraction dim w lands on SBUF partitions with 2KB-contiguous DMA runs.
  * Per pair: 3 PSUM banks qv_m = [q_m|v_m] (m=0,1) and kk = [k_m0|k_m1]. Biases
    are pre-loaded into PSUM via identity-matmuls (start=True), then 8 f32r data
    matmuls accumulate on top (f32r = 1 cycle/row at N>=256, ~1.7e-4 precision;
    inputs rounded to f32r via gpsimd cast-DMA / DVE copies).
  * Softmax: ScalarE evacuates k and v; a custom fused DVE op (TTR_MIN_NEG_ANT)
    computes s_neg = -(q*k) and the per-row -max in one pass; ScalarE Exp with
    per-partition bias and accumulated row sums; DVE reciprocal; one fused
    scalar_tensor_tensor computes out = (p * 1/sum) * v.  End-to-end rel err
    vs fp32 reference ~3.8e-3 (f32r matmul rounding).
"""
import sys

sys.path.insert(0, "/opt/trn_rl_repo")

import numpy as np

import concourse.bacc as bacc
import concourse.tile as tile
import concourse.dve_ops as dve_ops
from concourse import mybir
from concourse.masks import make_identity
from concourse.dve_spec import C0, C1, Spec, Src0, Src1, lower, minn, _has_src1
from concourse.dve_uop import DveOpSpec

F32 = mybir.dt.float32
F32R = mybir.dt.float32r

B = 8        # batch
C = 64       # channels total
CCH = 8      # channels per core
NCORES = 8
HP = 2       # h partition-tiles (h = 2j + m interleave)
KT = 2       # w partition-tiles (w = 2p + k interleave)
H = W = V = 256


def _make_ttr_min():
    """Custom DVE op: out = (in0*in1)*s1 ; accum_out = min(s0, row-min of out).
    Called with s1=-1, s0=+BIG: out = -(q*k), accum = -rowmax(q*k)."""
    name = "TTR_MIN_NEG_ANT"
    for op in dve_ops.OPS:
        if op.name == name:
            return op
    spec = Spec(
        body=Src0 * Src1 * C1,
        accum=minn,
        accum_init=C0,
        reference=lambda in0, in1, s0, s1, imm2: (
            np.asarray(in0, np.float32) * in1 * s1
        ),
    )
    row = dve_ops._CUSTOM_DVE_ROW_BASE + len(dve_ops.OPS)
    assert row < 0x20
    shas = {
        ver: DveOpSpec(name=name, opcode=row, uops=lower(spec, ver=ver),
                       rd1_en=_has_src1(spec)).sha(ver)
        for ver in ("v3", "v4")
    }
    op = dve_ops.DveOp(name, spec, subdim=False, uops_sha=shas)
    dve_ops.OPS.append(op)
    dve_ops.CUSTOM_DVE_SPECS[name] = spec
    dve_ops._SUB_OPCODE_FOR_NAME[name] = row
    return op


def _build_nc():
    ttr_min = _make_ttr_min()
    nc = bacc.Bacc("TRN2", target_bir_lowering=False, debug=False)
    xs = nc.dram_tensor("xs", [B, CCH, W, H], F32, kind="ExternalInput").ap()
    wq = nc.dram_tensor("wq", [CCH, W, V], F32, kind="ExternalInput").ap()
    wk = nc.dram_tensor("wk", [CCH, W, V], F32, kind="ExternalInput").ap()
    wv = nc.dram_tensor("wv", [CCH, W, V], F32, kind="ExternalInput").ap()
    bq = nc.dram_tensor("bq", [CCH, H, V], F32, kind="ExternalInput").ap()
    bk = nc.dram_tensor("bk", [CCH, H, V], F32, kind="ExternalInput").ap()
    bv = nc.dram_tensor("bv", [CCH, H, V], F32, kind="ExternalInput").ap()
    o = nc.dram_tensor("o", [B, CCH, H, V], F32, kind="ExternalOutput").ap()

    with tile.TileContext(nc) as tc:
        with (
            tc.tile_pool(name="const", bufs=1) as cpool,
            tc.tile_pool(name="wts", bufs=2) as wpool,
            tc.tile_pool(name="sb", bufs=3) as sb,
            tc.tile_pool(name="ps", bufs=2, space="PSUM") as ps,
        ):
            ident = cpool.tile([128, 128], F32)
            make_identity(nc, ident[:])
            ident_r = cpool.tile([128, 128], F32R)
            nc.vector.tensor_copy(ident_r[:], ident[:])

            for cc in range(CCH):
                # per-channel weights/biases loaded straight into f32r via
                # gpsimd cast-DMA (rounds during transfer). Rows interleaved
                # (w=2p+k / h=2p+m) -> all DMA runs 2KB contiguous.
                wqv_mm = wpool.tile([128, KT, 512], F32R, tag="wqv_r")
                nc.gpsimd.dma_start(wqv_mm[:, :, 0:V], wq[cc].rearrange("(p k) v -> p k v", k=KT))
                nc.gpsimd.dma_start(wqv_mm[:, :, V:2 * V], wv[cc].rearrange("(p k) v -> p k v", k=KT))
                wk_mm = wpool.tile([128, KT, V], F32R, tag="wk_r")
                nc.gpsimd.dma_start(wk_mm[:], wk[cc].rearrange("(p k) v -> p k v", k=KT))
                bqv_mm = wpool.tile([128, HP, 512], F32R, tag="bqv_r")
                nc.gpsimd.dma_start(bqv_mm[:, :, 0:V], bq[cc].rearrange("(p m) v -> p m v", m=HP))
                nc.gpsimd.dma_start(bqv_mm[:, :, V:2 * V], bv[cc].rearrange("(p m) v -> p m v", m=HP))
                bk_mm = wpool.tile([128, HP, V], F32R, tag="bk_r")
                nc.gpsimd.dma_start(bk_mm[:], bk[cc].rearrange("(p m) v -> p m v", m=HP))

                for b in range(B):
                    # xT load straight into f32r (gpsimd cast-DMA rounds)
                    xT = sb.tile([128, KT, H], F32R, tag="xT", bufs=4)
                    nc.gpsimd.dma_start(xT[:], xs[b, cc].rearrange("(p k) h -> p k h", k=KT))

                    # matmuls: bias preload (identity MM, start=True) + accumulate
                    qv_bank = [ps.tile([128, 512], F32, tag=f"qv{m}", name=f"qv{m}")
                               for m in range(HP)]
                    kk_bank = ps.tile([128, 512], F32, tag="kk")
                    for m in range(HP):
                        nc.tensor.matmul(qv_bank[m][:], ident_r[:], bqv_mm[:, m],
                                         start=True, stop=False)
                    nc.tensor.matmul(kk_bank[:], ident_r[:],
                                     bk_mm[:].rearrange("p m v -> p (m v)"),
                                     start=True, stop=False)
                    for m in range(HP):
                        for k in range(KT):
                            last = k == KT - 1
                            lq = xT[:, k, m * 128:(m + 1) * 128]
                            nc.tensor.matmul(qv_bank[m][:], lq, wqv_mm[:, k],
                                             start=False, stop=last,
                                             skip_group_check=True)
                            nc.tensor.matmul(kk_bank[:, m * 256:(m + 1) * 256], lq, wk_mm[:, k],
                                             start=False, stop=(last and m == HP - 1),
                                             skip_group_check=True)

                    # softmax chain
                    k_sb = sb.tile([128, 512], F32, tag="ksb")
                    nc.scalar.copy(k_sb[:], kk_bank[:])
                    v_sb = sb.tile([128, HP, 256], F32, tag="vsb")
                    for m in range(HP):
                        nc.scalar.copy(v_sb[:, m], qv_bank[m][:, 256:512])
                    s_sb = sb.tile([128, HP, 256], F32, tag="s")
                    mneg = sb.tile([128, HP], F32, tag="mneg")
                    for m in range(HP):
                        nc.vector._custom_dve(
                            ttr_min,
                            out=s_sb[:, m],
                            in0=qv_bank[m][:, 0:256],
                            in1=k_sb[:, m * 256:(m + 1) * 256],
                            s0=3.0e38, s1=-1.0,
                            accum_out=mneg[:, m:m + 1],
                        )
                    p_sb = sb.tile([128, HP, 256], F32, tag="p")
                    sums = sb.tile([128, HP], F32, tag="sums")
                    for m in range(HP):
                        nc.scalar.activation(
                            p_sb[:, m], s_sb[:, m],
                            mybir.ActivationFunctionType.Exp,
                            bias=mneg[:, m:m + 1], scale=-1.0,
                            accum_out=sums[:, m:m + 1],
                        )
                    r_sb = sb.tile([128, HP], F32, tag="r")
                    nc.vector.reciprocal(r_sb[:], sums[:])
                    out_sb = sb.tile([128, HP, 256], F32, tag="out", bufs=6)
                    for m in range(HP):
                        nc.vector.scalar_tensor_tensor(
                            out_sb[:, m], p_sb[:, m], r_sb[:, m:m + 1], v_sb[:, m],
                            op0=mybir.AluOpType.mult, op1=mybir.AluOpType.mult)
                    nc.sync.dma_start(o[b, cc].rearrange("(p m) v -> p m v", m=HP), out_sb[:])
    nc.compile()
    return nc


def _host_xT(xc):
    """[B, CC, H, W] -> xT [B, CC, W, H'] with H' enumerating h as f = m*128 + j
    <-> h = 2j + m (matches the kernel's interleaved row mapping)."""
    B_, C_, H_, W_ = xc.shape
    xt = xc.transpose(0, 1, 3, 2)
    xt = xt.reshape(B_, C_, W_, H_ // 2, 2).swapaxes(-1, -2)
    return np.ascontiguousarray(xt.reshape(B_, C_, W_, H_))


def kernel(x, query_w, key_w, var_w, query_b, key_b, var_b):
    from concourse.bass_utils import run_bass_kernel_spmd

    x = np.asarray(x, np.float32)
    in_maps = []
    for c in range(NCORES):
        sl = slice(c * CCH, (c + 1) * CCH)
        in_maps.append({
            "xs": _host_xT(x[:, sl]),
            "wq": np.ascontiguousarray(np.asarray(query_w, np.float32)[sl]),
            "wk": np.ascontiguousarray(np.asarray(key_w, np.float32)[sl]),
            "wv": np.ascontiguousarray(np.asarray(var_w, np.float32)[sl]),
            "bq": np.ascontiguousarray(np.asarray(query_b, np.float32)[sl]),
            "bk": np.ascontiguousarray(np.asarray(key_b, np.float32)[sl]),
            "bv": np.ascontiguousarray(np.asarray(var_b, np.float32)[sl]),
        })
    nc = _build_nc()
    res = run_bass_kernel_spmd(nc, in_maps, list(range(NCORES)))
    out = np.empty((B, C, H, V), np.float32)
    for c in range(NCORES):
        out[:, c * CCH:(c + 1) * CCH] = res.results[c]["o"]
    return out
